# revision 21
# baseline (speedup 1.0000x reference)
"""Trainium2 Bass kernel for nn_Attention_11424613007685 (v2).

Softcapped multi-head attention (H=16, HD=128, L=2048, D=2048, B=1):
  qkv = x @ w_qkv.T ; q,k RMSNorm (eps clamp) ; RoPE ; S = q k^T * scale ;
  softcap tanh(S/50)*50 ; softmax ; o = P@V ; y = o @ w_out.T

Sharding: heads tensor-parallel across 8 NeuronCores (2 heads/core). Each
core computes its heads' QKV projection from the full (transposed) input,
attention, and a partial output projection (row-sharded w_out); the host
sums the 8 fp16 partial outputs. All matmul operands are fp16 (host-cast);
accumulation is fp32 in PSUM; softmax statistics are fp32.

Structure: K^T and V for all L are computed first (PE-dense, quarter-major
x prefetch, host-pretransposed weights so all weight DMAs are contiguous),
then Q for the first L-half; attention (ACT-bound: tanh+exp) then runs per
(head, q-half) with the remaining Q quarters' projection matmuls AND their
rmsnorm/rope epilogues closure-scheduled into its kt loops (2 work items
per kt, so each quarter is fully finalized several iterations before the
half that consumes it - no boundary stall), plus the lt<8 output-projection
blocks. The softmax
denominator is accumulated elementwise on GPSIMD (frees ~65k PE cycles of
ones-matmuls) and inverted with the fast DVE reciprocal (staged through
SBUF - the custom op misreads PSUM); output partials are written fp16
(half the store traffic); x is SBUF-resident fp16 (read once).

Measured on TRN2 (NTFF profile): 321 us/core, rel err 9.6e-4.
"""

import sys

sys.path.insert(0, "/opt/trn_rl_repo")

from contextlib import ExitStack

import numpy as np

import concourse.bass as bass
import concourse.tile as tile
from concourse import bacc, bass_utils, mybir

F16 = mybir.dt.float16
F32 = mybir.dt.float32
AF = mybir.ActivationFunctionType
ALU = mybir.AluOpType

N_CORES = 8
B, L, D = 1, 2048, 2048
H, HD = 16, 128
HPC = H // N_CORES  # heads per core = 2
DPC = HPC * HD  # 256 per-core projection width
CAP = 50.0
EPS = 1e-6
QK_SCALE = HD**-0.5

NQ = 4  # l-quarters
QW = L // NQ  # 512
NCT = D // HD  # 16 c-tiles
KT = L // HD  # 16 k-tiles
HW_ = L // 2  # 1024 q-half width


def _build_nc(dbg: bool = False):
    nc = bacc.Bacc("TRN2", target_bir_lowering=False, debug=False,
                   num_devices=N_CORES)

    xT = nc.dram_tensor("xT", (D, L), F16, kind="ExternalInput").ap()
    wq = nc.dram_tensor("wq", (128, NCT, DPC), F16, kind="ExternalInput").ap()
    wk = nc.dram_tensor("wk", (128, NCT, DPC), F16, kind="ExternalInput").ap()
    wv = nc.dram_tensor("wv", (128, NCT, DPC), F16, kind="ExternalInput").ap()
    wo = nc.dram_tensor("wo", (HD, HPC, D), F16, kind="ExternalInput").ap()
    cosT = nc.dram_tensor("cosT", (HD, L), F16, kind="ExternalInput").ap()
    sinT = nc.dram_tensor("sinT", (HD, L), F16, kind="ExternalInput").ap()
    mperm = nc.dram_tensor("mperm", (HD, HD), F16, kind="ExternalInput").ap()
    gq = nc.dram_tensor("gq", (HD, 1), F32, kind="ExternalInput").ap()
    gk = nc.dram_tensor("gk", (HD, 1), F32, kind="ExternalInput").ap()
    yout = nc.dram_tensor("yout", (L, D), F16, kind="ExternalOutput").ap()
    if dbg:
        d_q16 = nc.dram_tensor("d_q16", (HD, L), F16, kind="ExternalOutput").ap()
        d_k16 = nc.dram_tensor("d_k16", (HD, L), F16, kind="ExternalOutput").ap()
        d_o = nc.dram_tensor("d_o", (HD, L), F16, kind="ExternalOutput").ap()
        d_v = nc.dram_tensor("d_v", (128, KT, DPC), F16,
                             kind="ExternalOutput").ap()
        d_acc = nc.dram_tensor("d_acc", (128, HW_), F16,
                               kind="ExternalOutput").ap()
        d_deni = nc.dram_tensor("d_deni", (1, HW_), F32,
                                kind="ExternalOutput").ap()

    coef = QK_SCALE / CAP

    with tile.TileContext(nc) as tc, ExitStack() as glb:
        # ---------------- global pools ----------------
        g_const = glb.enter_context(tc.tile_pool(name="g_const", bufs=1))
        g_x = glb.enter_context(tc.tile_pool(name="g_x", bufs=1))
        g_qk = glb.enter_context(tc.tile_pool(name="g_qk", bufs=1))
        g_misc = glb.enter_context(tc.tile_pool(name="g_misc", bufs=1))
        if dbg:
            g_dbg = glb.enter_context(tc.tile_pool(name="g_dbg", bufs=1))

        cos_sb = g_const.tile([HD, L], F16)
        sin_sb = g_const.tile([HD, L], F16)
        m_sb = g_const.tile([HD, HD], F16)
        gq_sb = g_const.tile([HD, 1], F32)
        gk_sb = g_const.tile([HD, 1], F32)
        ones_col = g_const.tile([128, 1], F16)
        ones_row = g_const.tile([1, 128], F16)

        # x resident: 16 tiles [128, L] fp16 (8 MB total)
        xts = [g_x.tile([128, L], F16, name=f"xt_{c}") for c in range(NCT)]
        wq_sb = g_x.tile([128, NCT, DPC], F16)
        wk_sb = g_x.tile([128, NCT, DPC], F16)
        wv_sb = g_x.tile([128, NCT, DPC], F16)
        wo_sb = g_x.tile([HD, HPC, D], F16)

        q16 = [g_qk.tile([HD, L], F16, name=f"q16_{h}") for h in range(HPC)]
        k16 = [g_qk.tile([HD, L], F16, name=f"k16_{h}") for h in range(HPC)]
        v_all = g_qk.tile([128, KT, DPC], F16)
        o16 = [g_qk.tile([HD, L], F16, name=f"o16_{h}") for h in range(HPC)]

        rs_rows = g_misc.tile([1, 2 * L], F32)
        rs16 = g_misc.tile([1, 2 * L], F16)
        rsk_sc = g_misc.tile([128, HPC * KT], F32)

        # DMA order: quarter-major x so the lq=0 K matmuls start after ~2MB
        nc.sync.dma_start(wk_sb[:, 0:4, :], wk[:, 0:4, :])
        nc.sync.dma_start(wv_sb[:, 0:4, :], wv[:, 0:4, :])
        nc.sync.dma_start(wk_sb[:, 4:NCT, :], wk[:, 4:NCT, :])
        nc.sync.dma_start(wv_sb[:, 4:NCT, :], wv[:, 4:NCT, :])
        nc.sync.dma_start(cos_sb[:], cosT)
        nc.sync.dma_start(sin_sb[:], sinT)
        nc.sync.dma_start(m_sb[:], mperm)
        nc.sync.dma_start(gq_sb[:], gq)
        nc.sync.dma_start(gk_sb[:], gk)
        nc.vector.memset(ones_col[:], 1.0)
        nc.vector.memset(ones_row[:], 1.0)
        for lq in range(NQ):
            ls = lq * QW
            for c in range(NCT):
                nc.sync.dma_start(xts[c][:, ls:ls + QW],
                                  xT[c * 128:(c + 1) * 128, ls:ls + QW])
            if lq == 0:
                nc.sync.dma_start(wq_sb[:], wq)
            if lq == 1:
                nc.sync.dma_start(wo_sb[:], wo)

        # shared attention-side SBUF pools
        b_t = glb.enter_context(tc.tile_pool(name="b_t", bufs=2))
        b_pp = glb.enter_context(tc.tile_pool(name="b_pp", bufs=4))
        b_acc = glb.enter_context(tc.tile_pool(name="b_acc", bufs=2))
        b_den = glb.enter_context(tc.tile_pool(name="b_den", bufs=2))
        c_st = glb.enter_context(tc.tile_pool(name="c_st", bufs=3))

        # ------------- pass 1: K^T and V for all L -------------
        with ExitStack() as sA:
            a_sq = sA.enter_context(tc.tile_pool(name="a_sq", bufs=3))
            a_qg = sA.enter_context(tc.tile_pool(name="a_qg", bufs=3))
            a_rope = sA.enter_context(tc.tile_pool(name="a_rope", bufs=2))
            a_ps = sA.enter_context(
                tc.tile_pool(name="a_ps", bufs=2, space="PSUM"))
            a_psv = sA.enter_context(
                tc.tile_pool(name="a_psv", bufs=1, space="PSUM"))
            a_psm = sA.enter_context(
                tc.tile_pool(name="a_psm", bufs=2, space="PSUM"))

            for lq in range(NQ):
                ls = lq * QW
                pk = [a_ps.tile([HD, QW], F32, name=f"pk_{h}", tag=f"pk{h}")
                      for h in range(HPC)]
                pv = [a_psv.tile([128, QW], F32, name=f"pv_{i}", tag="pv")
                      for i in range(2)]
                for c in range(NCT):
                    xt = xts[c][:, ls:ls + QW]
                    st, sp = c == 0, c == NCT - 1
                    for h in range(HPC):
                        nc.tensor.matmul(
                            pk[h][:], wk_sb[:, c, h * HD:(h + 1) * HD],
                            xt, start=st, stop=sp)
                    for j in range(4):
                        nc.tensor.matmul(
                            pv[j // 2][:, (j % 2) * DPC:(j % 2 + 1) * DPC],
                            xts[c][:, ls + j * 128:ls + (j + 1) * 128],
                            wv_sb[:, c, :], start=st and j % 2 == 0, stop=sp)

                for i in range(2):
                    nc.vector.tensor_copy(
                        v_all[:, lq * 4 + 2 * i:lq * 4 + 2 * i + 2, :],
                        pv[i][:])

                for h in range(HPC):
                    sq = a_sq.tile([HD, QW], F16, name="sq", tag="sq")
                    nc.scalar.activation(sq[:], pk[h][:], AF.Square, scale=1.0)
                    qgt = a_qg.tile([HD, QW], F16, name="qgt", tag="qg")
                    nc.vector.tensor_scalar_mul(qgt[:], pk[h][:], gk_sb[:])
                    pssk = a_psm.tile([128, 4], F32, name="pssk", tag="psm")
                    for j in range(4):
                        nc.tensor.matmul(
                            pssk[:, j:j + 1], sq[:, j * 128:(j + 1) * 128],
                            ones_col[:], start=j == 0, stop=j == 3)
                    kk = rsk_sc[:, h * KT + lq * 4:h * KT + lq * 4 + 4]
                    nc.vector.tensor_scalar(kk, pssk[:], 1.0 / HD, EPS,
                                            op0=ALU.mult, op1=ALU.max)
                    nc.scalar.activation(kk, kk, AF.Sqrt,
                                         scale=1.0 / (coef * coef))
                    nc.vector.reciprocal(kk, kk)
                    # rope on this quarter (K: final, scale folded into tanh)
                    t1 = a_rope.tile([HD, QW], F32, name="t1", tag="t1")
                    nc.gpsimd.tensor_mul(t1[:], qgt[:], cos_sb[:, ls:ls + QW])
                    pperm = a_psm.tile([HD, QW], F32, name="pperm", tag="psm")
                    nc.tensor.matmul(pperm[:], m_sb[:], qgt[:],
                                     start=True, stop=True)
                    t2 = a_rope.tile([HD, QW], F32, name="t2", tag="t2")
                    nc.vector.tensor_mul(t2[:], pperm[:], sin_sb[:, ls:ls + QW])
                    nc.gpsimd.tensor_add(k16[h][:, ls:ls + QW], t1[:], t2[:])

        # ------- pass 2a: Q^T for quarters 0,1 (2,3 interleave into attn) -----
        with ExitStack() as sQ:
            q_sq = sQ.enter_context(tc.tile_pool(name="q_sq", bufs=3))
            q_qg = sQ.enter_context(tc.tile_pool(name="q_qg", bufs=3))
            q_rope = sQ.enter_context(tc.tile_pool(name="q_rope", bufs=2))
            q_ps = sQ.enter_context(
                tc.tile_pool(name="q_ps", bufs=2, space="PSUM"))
            q_psm = sQ.enter_context(
                tc.tile_pool(name="q_psm", bufs=2, space="PSUM"))

            for lq in range(2):
                ls = lq * QW
                pq = [q_ps.tile([HD, QW], F32, name=f"pq_{h}", tag=f"pq{h}")
                      for h in range(HPC)]
                for c in range(NCT):
                    xt = xts[c][:, ls:ls + QW]
                    st, sp = c == 0, c == NCT - 1
                    for h in range(HPC):
                        nc.tensor.matmul(
                            pq[h][:], wq_sb[:, c, h * HD:(h + 1) * HD],
                            xt, start=st, stop=sp)
                for h in range(HPC):
                    sq = q_sq.tile([HD, QW], F16, name="sqq", tag="sq")
                    nc.scalar.activation(sq[:], pq[h][:], AF.Square, scale=1.0)
                    qgt = q_qg.tile([HD, QW], F16, name="qgtq", tag="qg")
                    nc.vector.tensor_scalar_mul(qgt[:], pq[h][:], gq_sb[:])
                    pssq = q_psm.tile([1, QW], F32, name="pssq", tag="psm")
                    nc.tensor.matmul(pssq[:], ones_col[:], sq[:],
                                     start=True, stop=True)
                    ro = lq * 2 * QW + h * QW
                    rr = rs_rows[:, ro:ro + QW]
                    nc.vector.tensor_scalar(rr, pssq[:], 1.0 / HD, EPS,
                                            op0=ALU.mult, op1=ALU.max)
                    # rope (unscaled q)
                    t1 = q_rope.tile([HD, QW], F32, name="t1q", tag="t1")
                    nc.gpsimd.tensor_mul(t1[:], qgt[:], cos_sb[:, ls:ls + QW])
                    pperm = q_psm.tile([HD, QW], F32, name="ppermq", tag="psm")
                    nc.tensor.matmul(pperm[:], m_sb[:], qgt[:],
                                     start=True, stop=True)
                    t2 = q_rope.tile([HD, QW], F32, name="t2q", tag="t2")
                    nc.vector.tensor_mul(t2[:], pperm[:], sin_sb[:, ls:ls + QW])
                    nc.gpsimd.tensor_add(q16[h][:, ls:ls + QW], t1[:], t2[:])
                # rms chain for both heads of this quarter, then scale q
                qrow = rs_rows[:, lq * 2 * QW:(lq + 1) * 2 * QW]
                nc.scalar.activation(qrow, qrow, AF.Sqrt, scale=1.0)
                nc.vector.reciprocal_approx_fast(qrow, qrow)
                nc.vector.tensor_copy(
                    rs16[:, lq * 2 * QW:(lq + 1) * 2 * QW], qrow)
                for h2 in range(HPC):
                    ro = lq * 2 * QW + h2 * QW
                    pbc = q_psm.tile([HD, QW], F32, name="pbc", tag="psm")
                    nc.tensor.matmul(pbc[:], ones_row[:],
                                     rs16[:, ro:ro + QW],
                                     start=True, stop=True)
                    nc.vector.tensor_mul(q16[h2][:, ls:ls + QW],
                                         q16[h2][:, ls:ls + QW], pbc[:])

        # ------------- pass 2b: attention + output projection -------------
        with ExitStack() as sB:
            s_ps = sB.enter_context(
                tc.tile_pool(name="s_ps", bufs=1, space="PSUM", side="right"))
            o_ps = sB.enter_context(
                tc.tile_pool(name="o_ps", bufs=1, space="PSUM"))
            aux_ps = sB.enter_context(
                tc.tile_pool(name="aux_ps", bufs=2, space="PSUM"))
            qi_ps = sB.enter_context(
                tc.tile_pool(name="qi_ps", bufs=1, space="PSUM"))
            qb_sq = sB.enter_context(tc.tile_pool(name="qb_sq", bufs=2))
            qb_qg = sB.enter_context(tc.tile_pool(name="qb_qg", bufs=2))
            qb_rope = sB.enter_context(tc.tile_pool(name="qb_rope", bufs=2))

            def q_step(pq, lq, c):
                ls = lq * QW
                st, sp = c == 0, c == NCT - 1
                for h in range(HPC):
                    nc.tensor.matmul(
                        pq[h][:], wq_sb[:, c, h * HD:(h + 1) * HD],
                        xts[c][:, ls:ls + QW], start=st, stop=sp)

            def q_epi_head(pq, lq, h):
                ls = lq * QW
                sq = qb_sq.tile([HD, QW], F16, name="sqi", tag="sq")
                nc.scalar.activation(sq[:], pq[h][:], AF.Square, scale=1.0)
                qgt = qb_qg.tile([HD, QW], F16, name="qgi", tag="qg")
                nc.vector.tensor_scalar_mul(qgt[:], pq[h][:], gq_sb[:])
                pssq = aux_ps.tile([1, QW], F32, name="pssqi", tag="aux")
                nc.tensor.matmul(pssq[:], ones_col[:], sq[:],
                                 start=True, stop=True)
                ro = lq * 2 * QW + h * QW
                rr = rs_rows[:, ro:ro + QW]
                nc.vector.tensor_scalar(rr, pssq[:], 1.0 / HD, EPS,
                                        op0=ALU.mult, op1=ALU.max)
                t1 = qb_rope.tile([HD, QW], F32, name="t1i", tag="t1")
                nc.gpsimd.tensor_mul(t1[:], qgt[:], cos_sb[:, ls:ls + QW])
                pperm = aux_ps.tile([HD, QW], F32, name="ppermi", tag="aux")
                nc.tensor.matmul(pperm[:], m_sb[:], qgt[:],
                                 start=True, stop=True)
                t2 = qb_rope.tile([HD, QW], F32, name="t2i", tag="t2")
                nc.vector.tensor_mul(t2[:], pperm[:], sin_sb[:, ls:ls + QW])
                nc.gpsimd.tensor_add(q16[h][:, ls:ls + QW], t1[:], t2[:])

            def q_epi_chain(lq):
                qrow = rs_rows[:, lq * 2 * QW:(lq + 1) * 2 * QW]
                nc.scalar.activation(qrow, qrow, AF.Sqrt, scale=1.0)
                nc.vector.reciprocal_approx_fast(qrow, qrow)
                nc.vector.tensor_copy(
                    rs16[:, lq * 2 * QW:(lq + 1) * 2 * QW], qrow)

            def q_epi_bc(lq, h2):
                ls = lq * QW
                ro = lq * 2 * QW + h2 * QW
                pbc = aux_ps.tile([HD, QW], F32, name="pbci", tag="aux")
                nc.tensor.matmul(pbc[:], ones_row[:], rs16[:, ro:ro + QW],
                                 start=True, stop=True)
                nc.vector.tensor_mul(q16[h2][:, ls:ls + QW],
                                     q16[h2][:, ls:ls + QW], pbc[:])

            def q_closures(pq, lq):
                cl = [(lambda c=c: q_step(pq, lq, c)) for c in range(NCT)]
                cl += [(lambda h=h: q_epi_head(pq, lq, h)) for h in range(HPC)]
                cl.append(lambda: q_epi_chain(lq))
                cl += [(lambda h=h: q_epi_bc(lq, h)) for h in range(HPC)]
                return cl

            def y_block(lt, use_act=False):
                """Output projection for l-tile lt: y[lt*128:, :] (both heads)."""
                for yc in range(4):
                    py = aux_ps.tile([128, 512], F32, name="py", tag="aux")
                    for h in range(HPC):
                        nc.tensor.matmul(
                            py[:], o16[h][:, lt * 128:(lt + 1) * 128],
                            wo_sb[:, h, yc * 512:(yc + 1) * 512],
                            start=h == 0, stop=h == HPC - 1)
                    stg = c_st.tile([128, 512], F16, name="stg", tag="stg")
                    if use_act and yc % 2 == 1:
                        nc.scalar.copy(stg[:], py[:])
                    else:
                        nc.vector.tensor_copy(stg[:], py[:])
                    nc.sync.dma_start(
                        yout[lt * 128:(lt + 1) * 128, yc * 512:(yc + 1) * 512],
                        stg[:])

            def attn_half(h, qh, y_pending, q_work=None):
                """kt loop over one q-half; y/q work interleaved per kt."""
                qs = qh * HW_
                ps_o = o_ps.tile([HD, HW_], F32, name="ps_o", tag="o")
                acc = b_acc.tile([128, HW_], F16, name="acc", tag="acc")
                for kt in range(KT):
                    ps_s = s_ps.tile([128, HW_], F32, name="ps_s", tag="s")
                    for i in range(2):
                        nc.tensor.matmul(
                            ps_s[:, i * 512:(i + 1) * 512],
                            k16[h][:, kt * 128:(kt + 1) * 128],
                            q16[h][:, qs + i * 512:qs + (i + 1) * 512],
                            start=True, stop=True)
                    tt = b_t.tile([128, HW_], F16, name="tt", tag="tt")
                    nc.scalar.activation(
                        tt[:], ps_s[:], AF.Tanh,
                        scale=rsk_sc[:, h * KT + kt:h * KT + kt + 1])
                    pp = b_pp.tile([128, HW_], F16, name="pp", tag="pp")
                    nc.scalar.activation(pp[:], tt[:], AF.Exp, scale=CAP,
                                         bias=0.0)
                    st, sp = kt == 0, kt == KT - 1
                    for i in range(2):
                        nc.tensor.matmul(
                            ps_o[:, i * 512:(i + 1) * 512],
                            v_all[:, kt, h * HD:(h + 1) * HD],
                            pp[:, i * 512:(i + 1) * 512], start=st, stop=sp)
                    if kt == 0:
                        nc.gpsimd.tensor_copy(acc[:], pp[:])
                    else:
                        nc.gpsimd.tensor_add(acc[:], acc[:], pp[:])
                    for _ in range(2):
                        if q_work:
                            q_work.pop(0)()
                    if y_pending and kt % 4 == 3:
                        y_block(y_pending.pop(0))
                # normalize: den = colsum(acc); o16 = ps_o / den
                pden = aux_ps.tile([64, 512], F32, name="pden", tag="aux")
                for i in range(2):
                    nc.tensor.matmul(pden[32 * i:32 * i + 1, :], ones_col[:],
                                     acc[:, i * 512:(i + 1) * 512],
                                     start=True, stop=True)
                for i in range(2):
                    qc = 2 * qh + i
                    cs = qc * 512
                    deni = b_den.tile([1, 512], F32, name="deni", tag="deni")
                    # recip_approx_fast misreads PSUM input (bitwise seed);
                    # stage the denominator through SBUF first
                    nc.vector.tensor_copy(deni[:], pden[32 * i:32 * i + 1, :])
                    nc.vector.reciprocal_approx_fast(deni[:], deni[:])
                    deni16 = b_den.tile([1, 512], F16, name="deni16",
                                        tag="deni16")
                    nc.vector.tensor_copy(deni16[:], deni[:])
                    pbcd = aux_ps.tile([HD, 512], F32, name="pbcd", tag="aux")
                    nc.tensor.matmul(pbcd[:], ones_row[:], deni16[:],
                                     start=True, stop=True)
                    bcd = b_den.tile([HD, 512], F16, name="bcd", tag="bcd")
                    nc.vector.tensor_copy(bcd[:], pbcd[:])
                    nc.vector.tensor_mul(o16[h][:, cs:cs + 512],
                                         ps_o[:, i * 512:(i + 1) * 512],
                                         bcd[:])
                    if dbg and h == 0 and qh == 0:
                        nc.sync.dma_start(d_deni[:, i * 512:(i + 1) * 512],
                                          deni[:])
                if dbg and h == 0 and qh == 0:
                    dacc = g_dbg.tile([128, HW_], F16, name="dacc")
                    nc.vector.tensor_copy(dacc[:], acc[:])
                    nc.sync.dma_start(d_acc, dacc[:])

            pq2 = [qi_ps.tile([HD, QW], F32, name=f"pqi_{h}",
                               tag=f"pqi{h}") for h in range(HPC)]
            attn_half(0, 0, [], q_work=q_closures(pq2, 2))
            pq3 = [qi_ps.tile([HD, QW], F32, name=f"pqj_{h}",
                               tag=f"pqi{h}") for h in range(HPC)]
            attn_half(1, 0, [], q_work=q_closures(pq3, 3))
            # y for lt 0-7 spread across the last two kt-loops
            attn_half(0, 1, [0, 1, 2, 3])
            attn_half(1, 1, [4, 5, 6, 7])

        with ExitStack() as sT:
            c_psW = sT.enter_context(
                tc.tile_pool(name="c_psW", bufs=2, space="PSUM"))
            for lt in range(8, KT):
                for half in range(2):
                    py = c_psW.tile([128, 1024], F32, name="pyw", tag="pyw")
                    ys = half * 1024
                    for yc in range(2):
                        for h in range(HPC):
                            nc.tensor.matmul(
                                py[:, yc * 512:(yc + 1) * 512],
                                o16[h][:, lt * 128:(lt + 1) * 128],
                                wo_sb[:, h,
                                      ys + yc * 512:ys + (yc + 1) * 512],
                                start=h == 0, stop=h == HPC - 1)
                    stg = c_st.tile([128, 1024], F16, name="stgw", tag="stgw")
                    if half:
                        nc.scalar.copy(stg[:], py[:])
                    else:
                        nc.vector.tensor_copy(stg[:], py[:])
                    nc.sync.dma_start(
                        yout[lt * 128:(lt + 1) * 128, ys:ys + 1024], stg[:])

        if dbg:
            for src_t, dst in ((q16[0], d_q16), (k16[0], d_k16),
                               (o16[0], d_o)):
                nc.sync.dma_start(dst, src_t[:])
            nc.sync.dma_start(d_v, v_all[:])

    nc.finalize()
    return nc


def _prep_inputs(x, cos, sin, w_qkv, w_out, q_gamma, k_gamma):
    x2 = np.asarray(x).reshape(L, D)
    xT16 = np.ascontiguousarray(x2.T).astype(np.float16)
    cosT = np.ascontiguousarray(np.asarray(cos).T).astype(np.float16)
    sinT = np.ascontiguousarray(np.asarray(sin).T).astype(np.float16)
    m = np.zeros((HD, HD), np.float16)
    half = HD // 2
    for d in range(half):
        m[d + half, d] = -1.0  # rh(x)[d] = -x[d+64], d < 64
    for d in range(half, HD):
        m[d - half, d] = 1.0  # rh(x)[d] = x[d-64], d >= 64
    gq = np.asarray(q_gamma).reshape(HD, 1).astype(np.float32)
    gk = np.asarray(k_gamma).reshape(HD, 1).astype(np.float32)
    w_qkv = np.asarray(w_qkv)
    w_out = np.asarray(w_out)

    in_maps = []
    for c in range(N_CORES):
        rows = np.concatenate(
            [np.arange((2 * c + h) * HD, (2 * c + h + 1) * HD)
             for h in range(HPC)])
        def _pcd(w):  # [D, DPC] -> [128, NCT, DPC] (partition-major)
            return np.ascontiguousarray(
                w.reshape(NCT, 128, DPC).transpose(1, 0, 2)).astype(np.float16)

        wq_c = _pcd(w_qkv[rows, :].T)
        wk_c = _pcd(w_qkv[D + rows, :].T)
        wv_c = _pcd(w_qkv[2 * D + rows, :].T)
        wo_c = np.ascontiguousarray(
            w_out[:, rows].T.reshape(HPC, HD, D).transpose(1, 0, 2)
        ).astype(np.float16)
        in_maps.append(dict(xT=xT16, wq=wq_c, wk=wk_c, wv=wv_c, wo=wo_c,
                            cosT=cosT, sinT=sinT, mperm=m, gq=gq, gk=gk))
    return in_maps


_CACHE = {}


def _run(in_maps, trace=False):
    if "nc" not in _CACHE:
        _CACHE["nc"] = _build_nc()
    nc = _CACHE["nc"]
    res = bass_utils.run_bass_kernel_spmd(
        nc, in_maps, core_ids=list(range(N_CORES)), trace=trace)
    y = np.zeros((L, D), np.float64)
    for r in res.results:
        y += r["yout"].astype(np.float64)
    return y.astype(np.float32).reshape(B, L, D), res


def kernel(x, cos, sin, w_qkv, w_out, q_gamma, k_gamma):
    in_maps = _prep_inputs(x, cos, sin, w_qkv, w_out, q_gamma, k_gamma)
    y, _ = _run(in_maps, trace=False)
    return y
